# revision 2
# baseline (speedup 1.0000x reference)
"""Causal MHA (B=4, T=2048, C=1024, H=16, D=64) on 8 trn2 cores — v2.

Sharding: core c = (batch c//2, head-group c%2 of 8 heads). Host sums the
two tensor-parallel partial y's per batch.

v2 changes vs baseline:
  - bf16 everywhere off the PE accumulators: xT, wqk, wv, qkT, attn,
    wproj (FWL weight loads, halved SBUF, no fp32r narrow-N penalty on
    diagonal ST blocks). Weights are downcast on-chip via DVE copies.
  - Phase D processed in head-PAIRS (2 heads in flight): the ST pair
    lands on PE row groups 0/64 (concurrent), ONE exp per (pair, kb)
    over a [128, 2, w] 3D AP (halves ACT instruction count), ONE
    pair-batched causal-mask multiply against a duplicated triangle.
  - PSUM budget: st 2x[128,2,512] (4 banks) + out 2x[65,512] (2 banks)
    + aux 2 banks (A/B/C/E tiles) = 8 banks.
  - C/D/E emitted interleaved per 512-column chunk so PE work from C/E
    fills the ACT-bound D windows: C(0); qc=0: D(0)+C(1); qc=1:
    D(1)+C(2)+E(0); qc=2: D(2)+C(3)+E(1); qc=3: D(3)+E(2); E(3).
  - A-phase psum->sbuf copies batched 4-wide, alternating DVE/ACT.
"""

import numpy as np

B, T, C, H, D = 4, 2048, 1024, 16, 64
HPG = 8            # heads per group (per core)
CG = HPG * D       # 512 features per group
SCALE = float(D) ** -0.5
NT = T // 128      # 16 T tiles
NKC = C // 128     # 8 contraction tiles over C
NQ = T // 512      # 4 Tq chunks
NMT = 8            # qkT feature tiles (1024 feats)

_PROG = None


def _build_program(loop_n=1, phases="ABCDE", dmode="full"):
    import contextlib
    import concourse.bacc as bacc
    import concourse.mybir as mybir
    import concourse.tile as tile
    from concourse.masks import make_identity

    F32 = mybir.dt.float32
    F32R = mybir.dt.float32r
    BF16 = mybir.dt.bfloat16
    Exp = mybir.ActivationFunctionType.Exp

    nc = bacc.Bacc("TRN2", target_bir_lowering=False, debug=False)

    with tile.TileContext(nc) as tc:
        loop_cm = tc.For_i(0, loop_n, 1) if loop_n > 1 \
            else contextlib.nullcontext()
        with loop_cm, \
             tc.tile_pool(name="dram", bufs=1, space="DRAM") as dram, \
             tc.tile_pool(name="persist", bufs=1) as persist, \
             tc.tile_pool(name="aux_ps", bufs=2, space="PSUM") as aux_ps, \
             tc.tile_pool(name="st_ps", bufs=2, space="PSUM") as st_ps, \
             tc.tile_pool(name="out_ps", bufs=2, space="PSUM") as out_ps, \
             tc.tile_pool(name="pt_pool", bufs=3) as pt_pool, \
             tc.tile_pool(name="small", bufs=3) as small, \
             tc.tile_pool(name="y_stage", bufs=3) as y_stage, \
             tc.tile_pool(name="x_pool", bufs=3) as x_pool:
            x_d = dram.tile([T, C], F32R, kind="ExternalInput", name="x",
                            uniquify=False)
            wqk_d = dram.tile([C, 2 * CG], F32R, kind="ExternalInput",
                              name="wqk", uniquify=False)
            wv_d = dram.tile([C, CG], F32R, kind="ExternalInput", name="wv",
                             uniquify=False)
            wproj_d = dram.tile([CG, C], F32R, kind="ExternalInput",
                                name="wproj", uniquify=False)
            y_d = dram.tile([T, C], F32, kind="ExternalOutput", name="y",
                            uniquify=False)

            # persistent SBUF
            qkT = persist.tile([128, NMT, T], BF16)       # 32 KB/part
            vaug = persist.tile([128, NT, HPG, D + 1], BF16)  # 16.25 KB
            attn = persist.tile([128, 4, T], BF16)        # 16 KB/part
            wproj_sb = persist.tile([128, 4, C], BF16)    # 8 KB/part
            wqk_sb = persist.tile([128, NKC, 2 * CG], BF16)  # 16 KB/part
            ident = persist.tile([128, 128], BF16)
            tri2 = persist.tile([128, 2, 128], BF16)      # causal triangle x2

            make_identity(nc, ident[:])
            # tri: keep[i, j] iff j - i >= 0, duplicated for head pairs
            for g in range(2):
                nc.vector.memset(tri2[:, g, :], 1.0)
                nc.gpsimd.affine_select(
                    out=tri2[:, g, :], in_=tri2[:, g, :],
                    compare_op=mybir.AluOpType.is_ge,
                    fill=0.0, base=0, channel_multiplier=-1,
                    pattern=[[1, 128]],
                )
            nc.vector.memset(vaug[:, :, :, D:D + 1], 1.0)

            dummy_pt = None
            if dmode == "stpv":
                dummy_pt = persist.tile([128, 2, 512], BF16)
                nc.vector.memset(dummy_pt[:], 0.001)

            with tc.tile_pool(name="wstage", bufs=1) as wstage:
                xT = wstage.tile([128, NKC, T], BF16)     # 32 KB/part
                wv_sb = wstage.tile([128, NKC, CG], BF16)  # 8 KB/part

                # staged fp32 loads, downcast on DVE. wv is needed by B(0)
                # so it loads upfront; wqk/wproj chunks stream inside the
                # A/B loop, hidden under compute.
                wqk_r = wqk_d[:].rearrange("(k p) n -> p k n", p=128)
                wv_r = wv_d[:].rearrange("(k p) n -> p k n", p=128)
                wproj_r = wproj_d[:].rearrange("(k p) n -> p k n", p=128)
                # prefetch the first x rows ahead of the wv staging DMAs
                x_rows = {}
                for tt in range(min(3, NT if "A" in phases else 0)):
                    xr = x_pool.tile([128, C], F32R, tag="xrow")
                    nc.sync.dma_start(
                        out=xr[:], in_=x_d[tt * 128:(tt + 1) * 128, :])
                    x_rows[tt] = xr

                for kc in range(NKC):
                    wstg = x_pool.tile([128, C], F32R, tag="wstg")
                    nc.sync.dma_start(out=wstg[:, 0:CG], in_=wv_r[:, kc, :])
                    nc.vector.tensor_copy(wv_sb[:, kc, :], wstg[:, 0:CG])

                def emit_w_chunk(i):
                    if i < NKC:
                        wstg = x_pool.tile([128, C], F32R, tag="wstg")
                        nc.sync.dma_start(out=wstg[:], in_=wqk_r[:, i, :])
                        nc.vector.tensor_copy(wqk_sb[:, i, :], wstg[:])
                    elif i < NKC + 4:
                        kt = i - NKC
                        wstg = x_pool.tile([128, C], F32R, tag="wstg")
                        nc.sync.dma_start(out=wstg[:], in_=wproj_r[:, kt, :])
                        nc.vector.tensor_copy(wproj_sb[:, kt, :], wstg[:])

                # ---------------- phases A+B (per T row block) -----------
                for tt in range(NT if "A" in phases else 0):
                    if tt in x_rows:
                        x_row = x_rows.pop(tt)
                    else:
                        x_row = x_pool.tile([128, C], F32R, tag="xrow")
                        nc.sync.dma_start(
                            out=x_row[:], in_=x_d[tt * 128:(tt + 1) * 128, :])
                    x_bf = x_pool.tile([128, C], BF16, tag="xbf")
                    nc.vector.tensor_copy(x_bf[:], x_row[:])
                    for half in range(2):
                        tp = st_ps.tile([128, 4, 128], BF16, tag="st")
                        for j in range(4):
                            kc = half * 4 + j
                            nc.tensor.transpose(
                                tp[:, j, :],
                                x_bf[:, kc * 128:(kc + 1) * 128],
                                ident[:])
                        dst = xT[:, half * 4:half * 4 + 4,
                                 tt * 128:(tt + 1) * 128]
                        if (tt + half) % 2 == 0:
                            nc.vector.tensor_copy(dst, tp[:])
                        else:
                            nc.scalar.copy(dst, tp[:])
                    if "B" in phases:
                        psv = out_ps.tile([128, CG], F32, tag="outp")
                        for kc in range(NKC):
                            nc.tensor.matmul(
                                psv[:],
                                xT[:, kc, tt * 128:(tt + 1) * 128],
                                wv_sb[:, kc, :],
                                start=(kc == 0), stop=(kc == NKC - 1))
                        nc.vector.tensor_copy(
                            vaug[:, tt, :, 0:D],
                            psv[:].rearrange("p (h d) -> p h d", h=HPG))
                    emit_w_chunk(tt)

                # ------------- C / D / E emission helpers ----------------
                def emit_c_m(n, m):
                    psq = aux_ps.tile([128, 512], F32, tag="aux")
                    for kc in range(NKC):
                        nc.tensor.matmul(
                            psq[:],
                            wqk_sb[:, kc, m * 128:(m + 1) * 128],
                            xT[:, kc, n * 512:(n + 1) * 512],
                            start=(kc == 0), stop=(kc == NKC - 1))
                    nc.vector.tensor_copy(
                        qkT[:, m, n * 512:(n + 1) * 512], psq[:])

                def emit_d_quarter(qc, quarter):
                    nkb = 4 * qc + 4
                    heads = (2 * quarter, 2 * quarter + 1)
                    outs = [out_ps.tile([D + 1, 512], F32, tag="outp",
                                        name=f"outp_{qc}_{h}")
                            for h in heads]
                    for kb in range(nkb):
                        r = kb - 4 * qc
                        jlo = 128 * r if r > 0 else 0
                        w = 512 - jlo
                        st = st_ps.tile([128, 2, 512], F32, tag="st",
                                        name=f"st_{qc}_{quarter}_{kb}")
                        for g, h in enumerate(heads):
                            pb = (h % 2) * 64
                            mq = h // 2
                            mk = 4 + h // 2
                            nc.tensor.matmul(
                                st[:, g, jlo:512],
                                qkT[pb:pb + 64, mk,
                                    kb * 128:(kb + 1) * 128],
                                qkT[pb:pb + 64, mq,
                                    qc * 512 + jlo:(qc + 1) * 512],
                                start=True, stop=True)
                        if dmode == "st":
                            continue
                        if dmode == "full":
                            pt = pt_pool.tile([128, 2, 512], BF16, tag="pt",
                                              name=f"pt_{qc}_{quarter}_{kb}")
                            nc.scalar.activation(
                                pt[:, :, jlo:512], st[:, :, jlo:512], Exp,
                                scale=SCALE)
                            if r >= 0:
                                nc.vector.tensor_mul(
                                    pt[:, :, jlo:jlo + 128],
                                    pt[:, :, jlo:jlo + 128],
                                    tri2[:])
                        else:
                            pt = dummy_pt
                        for g, h in enumerate(heads):
                            nc.tensor.matmul(
                                outs[g][:, jlo:512],
                                vaug[:, kb, h, :],
                                pt[:, g, jlo:512],
                                start=(kb == 0), stop=(kb == nkb - 1))
                    if dmode != "full":
                        return
                    for g, h in enumerate(heads):
                        pb = (h % 2) * 64
                        outp = outs[g]
                        rec = small.tile([1, 512], F32, tag="rec")
                        nc.vector.reciprocal(rec[:], outp[D:D + 1, :])
                        bc = small.tile([D, 512], F32, tag="bc")
                        nc.gpsimd.partition_broadcast(bc[:], rec[:])
                        nc.vector.tensor_mul(
                            attn[pb:pb + 64, h // 2,
                                 qc * 512:(qc + 1) * 512],
                            outp[0:D, :], bc[:])

                def emit_e_tt(tt):
                    for nn in range(2):
                        psy = aux_ps.tile([128, 512], F32, tag="aux")
                        for kt in range(4):
                            nc.tensor.matmul(
                                psy[:],
                                attn[:, kt, tt * 128:(tt + 1) * 128],
                                wproj_sb[:, kt, nn * 512:(nn + 1) * 512],
                                start=(kt == 0), stop=(kt == 3))
                        ys = y_stage.tile([128, 512], F32, tag="ys")
                        nc.vector.tensor_copy(ys[:], psy[:])
                        nc.sync.dma_start(
                            out=y_d[tt * 128:(tt + 1) * 128,
                                    nn * 512:(nn + 1) * 512],
                            in_=ys[:])

                # ------------- interleaved C/D/E schedule ----------------
                if "D" in phases:
                    if "C" in phases:
                        for m in range(NMT):
                            emit_c_m(0, m)
                    for qc in range(NQ):
                        for quarter in range(4):
                            emit_d_quarter(qc, quarter)
                            if "C" in phases and qc < 3:
                                emit_c_m(qc + 1, 2 * quarter)
                                emit_c_m(qc + 1, 2 * quarter + 1)
                            if "E" in phases and qc >= 1:
                                emit_e_tt(4 * (qc - 1) + quarter)
                    if "E" in phases:
                        for tt in range(8, NT):
                            emit_e_tt(tt)
                else:
                    if "C" in phases:
                        for n in range(NQ):
                            for m in range(NMT):
                                emit_c_m(n, m)
                    if "E" in phases:
                        for tt in range(NT):
                            emit_e_tt(tt)

    nc.compile()
    return nc


def _get_program():
    global _PROG
    if _PROG is None:
        _PROG = _build_program()
    return _PROG


def kernel(x, w_qkv, w_proj):
    from concourse.bass_utils import run_bass_kernel_spmd

    x = np.asarray(x, dtype=np.float32)
    w_qkv = np.asarray(w_qkv, dtype=np.float32)
    w_proj = np.asarray(w_proj, dtype=np.float32)

    in_maps = []
    for c in range(8):
        b, g = c // 2, c % 2
        wq = w_qkv[:, g * CG:(g + 1) * CG]
        wk = w_qkv[:, C + g * CG:C + (g + 1) * CG]
        wv = w_qkv[:, 2 * C + g * CG:2 * C + (g + 1) * CG]
        in_maps.append({
            "x": np.ascontiguousarray(x[b]),
            "wqk": np.ascontiguousarray(np.concatenate([wq, wk], axis=1)),
            "wv": np.ascontiguousarray(wv),
            "wproj": np.ascontiguousarray(w_proj[g * CG:(g + 1) * CG, :]),
        })

    nc = _get_program()
    res = run_bass_kernel_spmd(nc, in_maps, core_ids=list(range(8)))

    out = np.empty((B, T, C), dtype=np.float32)
    for b in range(B):
        out[b] = res.results[2 * b]["y"] + res.results[2 * b + 1]["y"]
    return out


# revision 4
# speedup vs baseline: 7.4895x; 7.4895x over previous
"""Causal MHA (B=4, T=2048, C=1024, H=16, D=64) on 8 trn2 cores.

Sharding: core c = (batch c//2, head-group c%2 of 8 heads). Host sums the
two tensor-parallel partial y's per batch (the w_proj row-shard reduce).

Per-core phases (all GEMM operands bf16 off the fp32 PSUM accumulators):
  A: transpose x [T,C] -> xT [C,T]: DMA fp32 row blocks, downcast to
     bf16 (DVE/ACT alternating), PE transpose, 4-wide psum->sbuf copies
     alternating DVE/ACT. wqk/wproj staging chunks stream inside this
     loop so their DMA hides under compute (wv loads upfront for B).
  B: vaug[T, 8x(64+1)] = x @ wv in bf16, ones column for the softmax
     denominators.
  C: qkT[feat, T] = (x @ wqk)^T via wqk^T @ x^T, written bf16.
  D: attention per (Tq chunk qc, head pair): the two ST matmuls sit at
     partition bases 0/64 of qkT so they land on disjoint PE row groups
     and run concurrently; ONE exp per (pair, kb) over a [128, 2, w]
     3D AP (halves ACT instruction count + per-instruction overhead);
     pair-batched causal-triangle mask; PV accumulates [65, 512] per
     head (row 64 = denominator); normalize into attn (bf16).
  E: y = attn^T @ wproj (bf16 weights -> fast weight load), fp32 out.

Schedule: C/D/E are emitted interleaved per 512-column chunk so PE work
from C/E fills the ACT-bound D windows: C(0); qc=0: D(0)+C(1); qc=1:
D(1)+C(2)+E(0); qc=2: D(2)+C(3)+E(1); qc=3: D(3)+E(2); E(3).
PSUM budget: st 2x[128,2,512] (4 banks) + out 2x[65,512] (2 banks) +
aux 2 banks (A/B/C/E accumulators) = 8 banks.

Measured (this session, interleaved loop-slope at 512/2048 iters):
~473 us/iter steady-state vs ~608 us/iter for the previous baseline
under the identical method (1.29x). Relative error vs fp32 reference:
3.1e-3 (tolerance 2e-2).
"""

import numpy as np

B, T, C, H, D = 4, 2048, 1024, 16, 64
HPG = 8            # heads per group (per core)
CG = HPG * D       # 512 features per group
SCALE = float(D) ** -0.5
NT = T // 128      # 16 T tiles
NKC = C // 128     # 8 contraction tiles over C
NQ = T // 512      # 4 Tq chunks
NMT = 8            # qkT feature tiles (1024 feats)

_PROG = None


def _build_program(loop_n=1, phases="ABCDE", dmode="full"):
    import contextlib
    import concourse.bacc as bacc
    import concourse.mybir as mybir
    import concourse.tile as tile
    from concourse.masks import make_identity

    F32 = mybir.dt.float32
    F32R = mybir.dt.float32r
    BF16 = mybir.dt.bfloat16
    Exp = mybir.ActivationFunctionType.Exp

    nc = bacc.Bacc("TRN2", target_bir_lowering=False, debug=False)

    with tile.TileContext(nc) as tc:
        loop_cm = tc.For_i(0, loop_n, 1) if loop_n > 1 \
            else contextlib.nullcontext()
        with loop_cm, \
             tc.tile_pool(name="dram", bufs=1, space="DRAM") as dram, \
             tc.tile_pool(name="persist", bufs=1) as persist, \
             tc.tile_pool(name="aux_ps", bufs=2, space="PSUM") as aux_ps, \
             tc.tile_pool(name="st_ps", bufs=2, space="PSUM") as st_ps, \
             tc.tile_pool(name="out_ps", bufs=2, space="PSUM") as out_ps, \
             tc.tile_pool(name="pt_pool", bufs=4) as pt_pool, \
             tc.tile_pool(name="small", bufs=3) as small, \
             tc.tile_pool(name="y_stage", bufs=3) as y_stage, \
             tc.tile_pool(name="x_pool", bufs=3) as x_pool:
            x_d = dram.tile([T, C], F32R, kind="ExternalInput", name="x",
                            uniquify=False)
            wqk_d = dram.tile([C, 2 * CG], F32R, kind="ExternalInput",
                              name="wqk", uniquify=False)
            wv_d = dram.tile([C, CG], F32R, kind="ExternalInput", name="wv",
                             uniquify=False)
            wproj_d = dram.tile([CG, C], F32R, kind="ExternalInput",
                                name="wproj", uniquify=False)
            y_d = dram.tile([T, C], F32, kind="ExternalOutput", name="y",
                            uniquify=False)

            # persistent SBUF
            qkT = persist.tile([128, NMT, T], BF16)       # 32 KB/part
            vaug = persist.tile([128, NT, HPG, D + 1], BF16)  # 16.25 KB
            attn = persist.tile([128, 4, T], BF16)        # 16 KB/part
            wproj_sb = persist.tile([128, 4, C], BF16)    # 8 KB/part
            wqk_sb = persist.tile([128, NKC, 2 * CG], BF16)  # 16 KB/part
            ident = persist.tile([128, 128], BF16)
            tri2 = persist.tile([128, 2, 128], BF16)      # causal triangle x2

            make_identity(nc, ident[:])
            # tri: keep[i, j] iff j - i >= 0, duplicated for head pairs
            for g in range(2):
                nc.vector.memset(tri2[:, g, :], 1.0)
                nc.gpsimd.affine_select(
                    out=tri2[:, g, :], in_=tri2[:, g, :],
                    compare_op=mybir.AluOpType.is_ge,
                    fill=0.0, base=0, channel_multiplier=-1,
                    pattern=[[1, 128]],
                )
            nc.vector.memset(vaug[:, :, :, D:D + 1], 1.0)

            dummy_pt = None
            if dmode == "stpv":
                dummy_pt = persist.tile([128, 2, 512], BF16)
                nc.vector.memset(dummy_pt[:], 0.001)

            with tc.tile_pool(name="wstage", bufs=1) as wstage:
                xT = wstage.tile([128, NKC, T], BF16)     # 32 KB/part
                wv_sb = wstage.tile([128, NKC, CG], BF16)  # 8 KB/part

                # staged fp32 loads, downcast on DVE. wv is needed by B(0)
                # so it loads upfront; wqk/wproj chunks stream inside the
                # A/B loop, hidden under compute.
                wqk_r = wqk_d[:].rearrange("(k p) n -> p k n", p=128)
                wv_r = wv_d[:].rearrange("(k p) n -> p k n", p=128)
                wproj_r = wproj_d[:].rearrange("(k p) n -> p k n", p=128)
                # prefetch the first x rows ahead of the wv staging DMAs
                x_rows = {}
                for tt in range(min(3, NT if "A" in phases else 0)):
                    xr = x_pool.tile([128, C], F32R, tag="xrow")
                    nc.sync.dma_start(
                        out=xr[:], in_=x_d[tt * 128:(tt + 1) * 128, :])
                    x_rows[tt] = xr

                for kc in range(NKC):
                    wstg = x_pool.tile([128, C], F32R, tag="wstg")
                    nc.sync.dma_start(out=wstg[:, 0:CG], in_=wv_r[:, kc, :])
                    if kc % 2 == 0:
                        nc.vector.tensor_copy(wv_sb[:, kc, :], wstg[:, 0:CG])
                    else:
                        nc.scalar.copy(wv_sb[:, kc, :], wstg[:, 0:CG])

                def emit_w_chunk(i):
                    if i < NKC:
                        wstg = x_pool.tile([128, C], F32R, tag="wstg")
                        nc.sync.dma_start(out=wstg[:], in_=wqk_r[:, i, :])
                        if i % 2 == 0:
                            nc.vector.tensor_copy(wqk_sb[:, i, :], wstg[:])
                        else:
                            nc.scalar.copy(wqk_sb[:, i, :], wstg[:])
                    elif i < NKC + 4:
                        kt = i - NKC
                        wstg = x_pool.tile([128, C], F32R, tag="wstg")
                        nc.sync.dma_start(out=wstg[:], in_=wproj_r[:, kt, :])
                        nc.vector.tensor_copy(wproj_sb[:, kt, :], wstg[:])

                # ---------------- phases A+B (per T row block) -----------
                for tt in range(NT if "A" in phases else 0):
                    if tt in x_rows:
                        x_row = x_rows.pop(tt)
                    else:
                        x_row = x_pool.tile([128, C], F32R, tag="xrow")
                        nc.sync.dma_start(
                            out=x_row[:], in_=x_d[tt * 128:(tt + 1) * 128, :])
                    x_bf = x_pool.tile([128, C], BF16, tag="xbf")
                    if tt % 2 == 0:
                        nc.vector.tensor_copy(x_bf[:], x_row[:])
                    else:
                        nc.scalar.copy(x_bf[:], x_row[:])
                    for half in range(2):
                        tp = st_ps.tile([128, 4, 128], BF16, tag="st")
                        for j in range(4):
                            kc = half * 4 + j
                            nc.tensor.transpose(
                                tp[:, j, :],
                                x_bf[:, kc * 128:(kc + 1) * 128],
                                ident[:])
                        dst = xT[:, half * 4:half * 4 + 4,
                                 tt * 128:(tt + 1) * 128]
                        if (tt + half) % 2 == 0:
                            nc.vector.tensor_copy(dst, tp[:])
                        else:
                            nc.scalar.copy(dst, tp[:])
                    if "B" in phases:
                        psv = out_ps.tile([128, CG], F32, tag="outp")
                        for kc in range(NKC):
                            nc.tensor.matmul(
                                psv[:],
                                xT[:, kc, tt * 128:(tt + 1) * 128],
                                wv_sb[:, kc, :],
                                start=(kc == 0), stop=(kc == NKC - 1))
                        nc.vector.tensor_copy(
                            vaug[:, tt, :, 0:D],
                            psv[:].rearrange("p (h d) -> p h d", h=HPG))
                    emit_w_chunk(tt)

                # ------------- C / D / E emission helpers ----------------
                def emit_c_m(n, m):
                    psq = aux_ps.tile([128, 512], F32, tag="aux")
                    for kc in range(NKC):
                        nc.tensor.matmul(
                            psq[:],
                            wqk_sb[:, kc, m * 128:(m + 1) * 128],
                            xT[:, kc, n * 512:(n + 1) * 512],
                            start=(kc == 0), stop=(kc == NKC - 1))
                    nc.vector.tensor_copy(
                        qkT[:, m, n * 512:(n + 1) * 512], psq[:])

                def emit_d_quarter(qc, quarter):
                    nkb = 4 * qc + 4
                    heads = (2 * quarter, 2 * quarter + 1)
                    outs = [out_ps.tile([D + 1, 512], F32, tag="outp",
                                        name=f"outp_{qc}_{h}")
                            for h in heads]
                    for kb in range(nkb):
                        r = kb - 4 * qc
                        jlo = 128 * r if r > 0 else 0
                        w = 512 - jlo
                        st = st_ps.tile([128, 2, 512], F32, tag="st",
                                        name=f"st_{qc}_{quarter}_{kb}")
                        for g, h in enumerate(heads):
                            pb = (h % 2) * 64
                            mq = h // 2
                            mk = 4 + h // 2
                            nc.tensor.matmul(
                                st[:, g, jlo:512],
                                qkT[pb:pb + 64, mk,
                                    kb * 128:(kb + 1) * 128],
                                qkT[pb:pb + 64, mq,
                                    qc * 512 + jlo:(qc + 1) * 512],
                                start=True, stop=True)
                        if dmode == "st":
                            continue
                        if dmode == "full":
                            pt = pt_pool.tile([128, 2, 512], BF16, tag="pt",
                                              name=f"pt_{qc}_{quarter}_{kb}")
                            nc.scalar.activation(
                                pt[:, :, jlo:512], st[:, :, jlo:512], Exp,
                                scale=SCALE)
                            if r >= 0:
                                nc.vector.tensor_mul(
                                    pt[:, :, jlo:jlo + 128],
                                    pt[:, :, jlo:jlo + 128],
                                    tri2[:])
                        else:
                            pt = dummy_pt
                        for g, h in enumerate(heads):
                            nc.tensor.matmul(
                                outs[g][:, jlo:512],
                                vaug[:, kb, h, :],
                                pt[:, g, jlo:512],
                                start=(kb == 0), stop=(kb == nkb - 1))
                    if dmode != "full":
                        return
                    for g, h in enumerate(heads):
                        pb = (h % 2) * 64
                        outp = outs[g]
                        rec = small.tile([1, 512], F32, tag="rec")
                        nc.vector.reciprocal(rec[:], outp[D:D + 1, :])
                        bc = small.tile([D, 512], F32, tag="bc")
                        nc.gpsimd.partition_broadcast(bc[:], rec[:])
                        nc.vector.tensor_mul(
                            attn[pb:pb + 64, h // 2,
                                 qc * 512:(qc + 1) * 512],
                            outp[0:D, :], bc[:])

                def emit_e_tt(tt):
                    for nn in range(2):
                        psy = aux_ps.tile([128, 512], F32, tag="aux")
                        for kt in range(4):
                            nc.tensor.matmul(
                                psy[:],
                                attn[:, kt, tt * 128:(tt + 1) * 128],
                                wproj_sb[:, kt, nn * 512:(nn + 1) * 512],
                                start=(kt == 0), stop=(kt == 3))
                        ys = y_stage.tile([128, 512], F32, tag="ys")
                        # tail tt's copy on ACT (idle once exps are done)
                        if tt >= 12:
                            nc.scalar.copy(ys[:], psy[:])
                        else:
                            nc.vector.tensor_copy(ys[:], psy[:])
                        nc.sync.dma_start(
                            out=y_d[tt * 128:(tt + 1) * 128,
                                    nn * 512:(nn + 1) * 512],
                            in_=ys[:])

                # ------------- interleaved C/D/E schedule ----------------
                if "D" in phases:
                    if "C" in phases:
                        for m in range(NMT):
                            emit_c_m(0, m)
                    for qc in range(NQ):
                        for quarter in range(4):
                            emit_d_quarter(qc, quarter)
                            if "C" in phases and qc < 3:
                                emit_c_m(qc + 1, 2 * quarter)
                                emit_c_m(qc + 1, 2 * quarter + 1)
                            if "E" in phases and qc >= 1:
                                emit_e_tt(4 * (qc - 1) + quarter)
                    if "E" in phases:
                        for tt in range(8, NT):
                            emit_e_tt(tt)
                else:
                    if "C" in phases:
                        for n in range(NQ):
                            for m in range(NMT):
                                emit_c_m(n, m)
                    if "E" in phases:
                        for tt in range(NT):
                            emit_e_tt(tt)

    nc.compile()
    return nc


def _get_program():
    global _PROG
    if _PROG is None:
        _PROG = _build_program()
    return _PROG


def kernel(x, w_qkv, w_proj):
    from concourse.bass_utils import run_bass_kernel_spmd

    x = np.asarray(x, dtype=np.float32)
    w_qkv = np.asarray(w_qkv, dtype=np.float32)
    w_proj = np.asarray(w_proj, dtype=np.float32)

    in_maps = []
    for c in range(8):
        b, g = c // 2, c % 2
        wq = w_qkv[:, g * CG:(g + 1) * CG]
        wk = w_qkv[:, C + g * CG:C + (g + 1) * CG]
        wv = w_qkv[:, 2 * C + g * CG:2 * C + (g + 1) * CG]
        in_maps.append({
            "x": np.ascontiguousarray(x[b]),
            "wqk": np.ascontiguousarray(np.concatenate([wq, wk], axis=1)),
            "wv": np.ascontiguousarray(wv),
            "wproj": np.ascontiguousarray(w_proj[g * CG:(g + 1) * CG, :]),
        })

    nc = _get_program()
    res = run_bass_kernel_spmd(nc, in_maps, core_ids=list(range(8)))

    out = np.empty((B, T, C), dtype=np.float32)
    for b in range(B):
        out[b] = res.results[2 * b]["y"] + res.results[2 * b + 1]["y"]
    return out
